# revision 30
# baseline (speedup 1.0000x reference)
"""Trainium2 Bass kernel for FlattenIntraCycleMoELayer (top-2 MoE + general path).

Strategy:
  - Data-parallel over B (8 batteries per core).
  - Gating MLP layer-1 is d_ff-sharded across the 8 cores (fp32 weights for
    selection fidelity); partial logits are AllReduced (2KB collective).
  - Top-2 selection via DVE Max8/MaxIndex; the gate-weighted expert combine
    reads the resident expert block with runtime expert indices
    (reg_load + dynamic slice); biases are folded into an appended ones-row
    of the contraction dim.
  - Main matmuls (general + moe) run in bf16 (1 cyc/row) with fp32 PSUM
    accumulation; the moe output is rounded to bf16 before the final add,
    matching the reference.
  - The AllReduce latency is hidden by running the gate-independent general
    matmuls of the first NSTAGE batteries first (staged to SBUF).

Host-side prep only reshapes/pads/casts inputs (no model math on host).
"""

import numpy as np
import ml_dtypes


def _ensure_import_path():
    try:
        import concourse  # noqa: F401
    except ImportError:
        import sys
        for p in ("/opt/trn_rl_repo", "/root/.axon_site/_ro/trn_rl_repo"):
            if p not in sys.path:
                sys.path.insert(0, p)
        import concourse  # noqa: F401


_ensure_import_path()

import concourse.bass as bass  # noqa: E402
import concourse.tile as tile  # noqa: E402
from concourse import mybir  # noqa: E402
from concourse.bass import ds, ts  # noqa: E402
from concourse.alu_op_type import AluOpType  # noqa: E402
from concourse.masks import make_identity  # noqa: E402
from concourse.tile import add_dep_helper  # noqa: E402

BF16 = mybir.dt.bfloat16
F32 = mybir.dt.float32
U32 = mybir.dt.uint32
F32R = mybir.dt.float32r

# Problem shape constants (hardcoded per contest rules).
B, L, C, F = 64, 512, 3, 300
CF = C * F              # 900
KP = 1024               # padded contraction dim (900 data + 1 ones + 123 zero)
KT = KP // 128          # 8 k-tiles
D = 512                 # d_model
E = 8                   # experts
NCORES = 8
BPC = B // NCORES       # 8 batteries per core
DLLM = 4096
GK = 4224               # padded gating contraction = 33*128
GKT = GK // 128         # 33
DFF = 2048
DFFC = DFF // NCORES    # 256 per-core d_ff chunk
EPS = 1e-9

MT = L // 128           # 4 m-tiles per battery
NSTAGE = 3              # batteries whose general path runs before gates arrive


def build_program(nc):
    from contextlib import ExitStack

    xt = nc.dram_tensor("xt", [BPC, KP, L], BF16, kind="ExternalInput")
    ew = nc.dram_tensor("ew", [E, KP, D], BF16, kind="ExternalInput")
    gw = nc.dram_tensor("gw", [KP, D], BF16, kind="ExternalInput")
    gint = nc.dram_tensor("gint", [GK, B], F32, kind="ExternalInput")
    w1c = nc.dram_tensor("w1c", [GK, DFFC], F32, kind="ExternalInput")
    w2c = nc.dram_tensor("w2c", [DFFC, E], F32, kind="ExternalInput")
    selt = nc.dram_tensor("selt", [B, BPC], F32, kind="ExternalInput")
    b2 = nc.dram_tensor("b2", [1, E], F32, kind="ExternalInput")
    out = nc.dram_tensor("out", [BPC, L, D], F32, kind="ExternalOutput")

    xt_ap = xt.ap()
    out_ap = out.ap()

    with tile.TileContext(nc) as tc, ExitStack() as ctx:
        singles = ctx.enter_context(tc.tile_pool(name="singles", bufs=1))
        gpool = ctx.enter_context(tc.tile_pool(name="gate", bufs=1))
        xpool = ctx.enter_context(tc.tile_pool(name="xts", bufs=NSTAGE + 1))
        wbpool = ctx.enter_context(tc.tile_pool(name="wbs", bufs=3))
        scpool = ctx.enter_context(tc.tile_pool(name="scratch", bufs=2))
        opool = ctx.enter_context(tc.tile_pool(name="outs", bufs=3))
        dpool = ctx.enter_context(tc.tile_pool(name="dram", bufs=1, space="DRAM"))
        mps = ctx.enter_context(tc.tile_pool(name="mpsum", bufs=2, space="PSUM"))
        gps_ctx = ExitStack()
        gps = gps_ctx.enter_context(tc.tile_pool(name="gpsum", bufs=1, space="PSUM"))
        w1_ctx = ExitStack()
        w1pool = w1_ctx.enter_context(tc.tile_pool(name="w1s", bufs=1))

        # ---------------- Phase 1: gating (DMAs first, highest priority) ----
        ginT_sb = gpool.tile([128, GKT, B], F32)
        nc.sync.dma_start(
            out=ginT_sb, in_=gint.ap().rearrange("(kt p) b -> p kt b", p=128)
        )
        w2_sb = gpool.tile([128, DFFC // 128, E], F32)
        nc.sync.dma_start(
            out=w2_sb, in_=w2c.ap().rearrange("(j p) e -> p j e", p=128)
        )
        selt_sb = gpool.tile([B, BPC], F32)
        nc.sync.dma_start(out=selt_sb, in_=selt.ap())
        b2_ap = b2.ap()
        b2bc = gpool.tile([BPC, E], F32)
        nc.gpsimd.dma_start(
            out=b2bc,
            in_=bass.AP(tensor=b2_ap.tensor, offset=b2_ap.offset,
                        ap=[[0, BPC]] + list(b2_ap.ap[1:])),
        )
        ident = singles.tile([128, 128], F32)
        make_identity(nc, ident)

        # layer 1: h_chunk = gelu(g_in @ W1[:, chunk])   [B, DFFC] fp32
        # W1 resident, loaded via two parallel HWDGE streams (SP + ACT issue)
        w1_sb = w1pool.tile([128, GKT, DFFC], F32)
        w1ap = w1c.ap().rearrange("(kt p) f -> p kt f", p=128)
        half = GKT // 2
        nc.sync.dma_start(out=w1_sb[:, :half, :], in_=w1ap[:, :half, :])
        w1_dma = nc.scalar.dma_start(out=w1_sb[:, half:, :], in_=w1ap[:, half:, :])
        psum_h = gps.tile([B, DFFC], F32, bufs=1)
        for kt in range(GKT):
            nc.tensor.matmul(
                out=psum_h, lhsT=ginT_sb[:, kt, :], rhs=w1_sb[:, kt, :],
                start=(kt == 0), stop=(kt == GKT - 1),
            )
        w1_ctx.close()  # release W1 buffer
        stpool = ctx.enter_context(tc.tile_pool(name="stage", bufs=1))
        # gelu (tanh approx) from primitives:
        #   h = 0.5*x*(1 + tanh(0.79788456*(x + 0.044715*x^3)))
        g_x = gpool.tile([B, DFFC], F32)
        nc.vector.tensor_copy(out=g_x, in_=psum_h)
        g_x2 = gpool.tile([B, DFFC], F32)
        nc.vector.tensor_tensor(out=g_x2, in0=g_x, in1=g_x, op=AluOpType.mult)
        g_p = gpool.tile([B, DFFC], F32)
        nc.vector.tensor_scalar(g_p, g_x2, 0.044715, 1.0,
                                AluOpType.mult, AluOpType.add)
        g_u = gpool.tile([B, DFFC], F32)
        nc.vector.tensor_tensor(out=g_u, in0=g_x, in1=g_p, op=AluOpType.mult)
        g_t = gpool.tile([B, DFFC], F32)
        nc.scalar.activation(out=g_t, in_=g_u,
                             func=mybir.ActivationFunctionType.Tanh,
                             scale=0.7978845608028654)
        g_q = gpool.tile([B, DFFC], F32)
        nc.vector.tensor_scalar(g_q, g_t, 1.0, 0.5,
                                AluOpType.add, AluOpType.mult)
        h_sb = gpool.tile([B, DFFC], F32)
        nc.vector.tensor_tensor(out=h_sb, in0=g_x, in1=g_q, op=AluOpType.mult)

        # transpose h chunk -> hT [128, j, B]
        hT_sb = gpool.tile([128, DFFC // 128, B], F32)
        for j in range(DFFC // 128):
            pst = gps.tile([128, B], F32, bufs=2, tag="pst")
            nc.tensor.transpose(
                out=pst, in_=h_sb[:, j * 128:(j + 1) * 128], identity=ident[:B, :B]
            )
            nc.vector.tensor_copy(out=hT_sb[:, j, :], in_=pst)

        # layer 2 partial logits for ALL batteries: [B, E]
        psum_l = gps.tile([B, E], F32, bufs=2, tag="pst")
        for j in range(DFFC // 128):
            nc.tensor.matmul(
                out=psum_l, lhsT=hT_sb[:, j, :], rhs=w2_sb[:, j, :],
                start=(j == 0), stop=(j == DFFC // 128 - 1),
            )
        pl_sb = gpool.tile([B, E], F32)
        nc.vector.tensor_copy(out=pl_sb, in_=psum_l)

        # AllReduce partial logits across the 8 cores.
        import os as _os
        ar_in = dpool.tile([B, E], F32)
        nc.gpsimd.dma_start(out=ar_in, in_=pl_sb)
        logits_all = gpool.tile([B, E], F32)
        if _os.environ.get("MOE_NO_AR"):
            nc.gpsimd.dma_start(out=logits_all, in_=ar_in)
        else:
            ar_out = dpool.tile([B, E], F32, addr_space="Shared")
            nc.gpsimd.collective_compute(
                "AllReduce", AluOpType.add,
                replica_groups=[list(range(NCORES))],
                ins=[ar_in], outs=[ar_out],
            )
            nc.gpsimd.dma_start(out=logits_all, in_=ar_out)

        # select this core's 8 batteries via one-hot matmul, add gate_b2
        psum_sel = gps.tile([BPC, E], F32, bufs=2, tag="pst")
        nc.tensor.matmul(out=psum_sel, lhsT=selt_sb, rhs=logits_all,
                         start=True, stop=True)
        logits_my = gpool.tile([BPC, E], F32)
        nc.vector.tensor_tensor(out=logits_my, in0=psum_sel, in1=b2bc,
                                op=AluOpType.add)
        gps_ctx.close()  # release gating PSUM banks

        # top-2 gates: sorted values + indices, softmax renorm on top-2
        sorted8 = gpool.tile([BPC, E], F32)
        sidx = gpool.tile([BPC, E], U32)
        nc.vector.max(out=sorted8, in_=logits_my)
        nc.vector.max_index(out=sidx, in_max=sorted8, in_values=logits_my)
        negmax = gpool.tile([BPC, 1], F32)
        nc.vector.tensor_scalar_mul(negmax, sorted8[:, 0:1], -1.0)
        q = gpool.tile([BPC, E], F32)
        nc.scalar.activation(out=q, in_=sorted8,
                             func=mybir.ActivationFunctionType.Exp,
                             bias=negmax, scale=1.0)
        zsum = gpool.tile([BPC, 1], F32)
        nc.vector.reduce_sum(out=zsum, in_=q, axis=mybir.AxisListType.X)
        t12 = gpool.tile([BPC, 1], F32)
        nc.vector.tensor_tensor(out=t12, in0=q[:, 0:1], in1=q[:, 1:2],
                                op=AluOpType.add)
        den = gpool.tile([BPC, 1], F32)
        nc.vector.scalar_tensor_tensor(out=den, in0=zsum, scalar=EPS, in1=t12,
                                       op0=AluOpType.mult, op1=AluOpType.add)
        rden = gpool.tile([BPC, 1], F32)
        nc.vector.reciprocal(out=rden, in_=den)
        g12 = gpool.tile([BPC, 2], F32)
        nc.vector.tensor_scalar_mul(g12, q[:, 0:2], rden)

        # broadcast (g1,g2) of each battery to all 128 partitions (via DRAM):
        # [8,2] --fold--> [1,16] --bcast--> [128,16], in both f32 and bf16
        gflat = gpool.tile([1, 2 * BPC], F32)
        nc.gpsimd.dma_start(out=gflat, in_=g12)
        gflat_bf = gpool.tile([1, 2 * BPC], BF16)
        nc.vector.tensor_copy(out=gflat_bf, in_=gflat)
        gflat_dram = dpool.tile([1, 2 * BPC], F32)
        nc.gpsimd.dma_start(out=gflat_dram, in_=gflat)
        gflat_bf_dram = dpool.tile([1, 2 * BPC], BF16)
        nc.gpsimd.dma_start(out=gflat_bf_dram, in_=gflat_bf)
        bcastG = gpool.tile([128, 2 * BPC], F32)
        nc.gpsimd.dma_start(
            out=bcastG,
            in_=bass.AP(tensor=gflat_dram.tensor, offset=gflat_dram.offset,
                        ap=[[0, 128]] + list(gflat_dram.ap[1:])),
        )
        bcastGbf = gpool.tile([128, 2 * BPC], BF16)
        nc.gpsimd.dma_start(
            out=bcastGbf,
            in_=bass.AP(tensor=gflat_bf_dram.tensor, offset=gflat_bf_dram.offset,
                        ap=[[0, 128]] + list(gflat_bf_dram.ap[1:])),
        )

        # ---------------- Phase 2: main weights (after gating DMAs) --------
        # DMA priority: gating (above) -> GW -> xb[0..NSTAGE-1] -> EXP -> rest
        GW_sb = singles.tile([128, KT, D], BF16)
        gw_dma = nc.sync.dma_start(
            out=GW_sb, in_=gw.ap().rearrange("(kt p) d -> p kt d", p=128)
        )
        add_dep_helper(gw_dma.ins, w1_dma.ins, sync=False,
                       reason="order GW stream after gating W1")

        xb_tiles = {}

        def load_xb(b, after=None):
            xb = xpool.tile([128, KT, L], BF16)
            dma = nc.sync.dma_start(
                out=xb, in_=xt_ap[b].rearrange("(kt p) l -> p kt l", p=128)
            )
            if after is not None:
                add_dep_helper(dma.ins, after.ins, sync=False,
                               reason="xb DMA ordering")
            xb_tiles[b] = xb
            return dma

        prev = gw_dma
        for b in range(NSTAGE):
            prev = load_xb(b, after=prev)

        EXP_sb = singles.tile([128, E, KT, D], BF16)
        exp_dma = nc.sync.dma_start(
            out=EXP_sb, in_=ew.ap().rearrange("e (kt p) d -> p e kt d", p=128)
        )
        add_dep_helper(exp_dma.ins, prev.ins, sync=False,
                       reason="order EXP stream after staged xb")

        def _vload(eng, ap, name):
            reg = eng.alloc_register(name)
            eng.reg_load(reg, ap)
            val = eng.snap(reg, donate=True)
            return nc.s_assert_within(val, 0, E - 1, skip_runtime_assert=True)

        def combine(b, fused=False):
            """wb = g1*W_e1 + g2*W_e2 (+ gen_W when fused) for battery b."""
            rv1 = _vload(nc.scalar, sidx[b:b + 1, 0:1], f"e1_{b}")
            rv2 = _vload(nc.vector, sidx[b:b + 1, 1:2], f"e2_{b}")
            wb = wbpool.tile([128, KT, D], BF16)
            tsc = scpool.tile([128, KT, D], BF16, tag="tsc")
            nc.scalar.activation(
                out=tsc.rearrange("p k d -> p (k d)"),
                in_=EXP_sb[:, ds(rv1, 1), :, :].rearrange("p o k d -> p (o k d)"),
                func=mybir.ActivationFunctionType.Copy,
                scale=bcastG[:, 2 * b:2 * b + 1],
            )
            if fused:
                # wb = (g2*E2 + GW) + tsc   -> single fused weight matrix
                nc.vector.scalar_tensor_tensor(
                    out=wb,
                    in0=EXP_sb[:, ds(rv2, 1), :, :].rearrange("p o k d -> p (o k) d"),
                    scalar=bcastGbf[:, 2 * b + 1:2 * b + 2],
                    in1=GW_sb,
                    op0=AluOpType.mult, op1=AluOpType.add,
                )
                nc.vector.tensor_tensor(out=wb, in0=wb, in1=tsc,
                                        op=AluOpType.add)
            else:
                nc.vector.scalar_tensor_tensor(
                    out=wb,
                    in0=EXP_sb[:, ds(rv2, 1), :, :].rearrange("p o k d -> p (o k) d"),
                    scalar=bcastGbf[:, 2 * b + 1:2 * b + 2],
                    in1=tsc,
                    op0=AluOpType.mult, op1=AluOpType.add,
                )
            return wb

        def general_mms(b, xb, dst_psums):
            for m in range(MT):
                pg = mps.tile([128, D], F32, tag="pg", bufs=2)
                for kt in range(KT):
                    nc.tensor.matmul(
                        out=pg, lhsT=xb[:, kt, ts(m, 128)], rhs=GW_sb[:, kt, :],
                        start=(kt == 0), stop=(kt == KT - 1),
                    )
                dst_psums.append(pg)

        def moe_and_evict(b, xb, wb, staged=None, pg_list=None):
            for m in range(MT):
                pm = mps.tile([128, D], F32, tag="pm", bufs=2)
                for kt in range(KT):
                    nc.tensor.matmul(
                        out=pm, lhsT=xb[:, kt, ts(m, 128)], rhs=wb[:, kt, :],
                        start=(kt == 0), stop=(kt == KT - 1),
                    )
                moebf = scpool.tile([128, D], BF16, tag="moebf")
                nc.scalar.activation(out=moebf, in_=pm,
                                     func=mybir.ActivationFunctionType.Copy)
                osb = opool.tile([128, D], F32)
                gen_src = staged[m] if staged is not None else pg_list[m]
                nc.vector.tensor_tensor(out=osb, in0=gen_src, in1=moebf,
                                        op=AluOpType.add)
                nc.sync.dma_start(out=out_ap[b, ts(m, 128), :], in_=osb)

        # --- staged batteries: general first (runs during the AllReduce) ---
        staged_gen = {}
        for b in range(NSTAGE):
            pgs = []
            general_mms(b, xb_tiles[b], pgs)
            stg = []
            for m in range(MT):
                st = stpool.tile([128, D], F32, tag=f"stage_{b}_{m}")
                nc.vector.tensor_copy(out=st, in_=pgs[m])
                stg.append(st)
            staged_gen[b] = stg

        # --- gated work: combines emitted two batteries ahead ---
        def emit_battery(b):
            xb = xb_tiles[b]
            wb = wbs[b]
            if b < NSTAGE:
                moe_and_evict(b, xb, wb, staged=staged_gen[b])
                return
            # fused single matmul: out = x @ (gen_W + g1*E1 + g2*E2)
            for m in range(MT):
                pf = mps.tile([128, D], F32, tag="pg", bufs=2)
                for kt in range(KT):
                    nc.tensor.matmul(
                        out=pf, lhsT=xb[:, kt, ts(m, 128)], rhs=wb[:, kt, :],
                        start=(kt == 0), stop=(kt == KT - 1),
                    )
                osb = opool.tile([128, D], F32)
                nc.vector.tensor_copy(out=osb, in_=pf)
                nc.sync.dma_start(out=out_ap[b, ts(m, 128), :], in_=osb)

        wbs = {}
        for i in range(min(3, BPC)):
            wbs[i] = combine(i, fused=(i >= NSTAGE))
        for b in range(BPC):
            nxt = b + 3
            if nxt < BPC:
                if nxt >= NSTAGE:
                    load_xb(nxt)
                wbs[nxt] = combine(nxt, fused=(nxt >= NSTAGE))
            elif b + 2 == NSTAGE + 4 and False:
                pass
            emit_battery(b)


def make_nc():
    from concourse import bacc
    nc = bacc.Bacc("TRN2", target_bir_lowering=False, debug=False,
                   num_devices=NCORES)
    build_program(nc)
    nc.finalize()
    return nc


def prep_inputs(cycle_curve_data, cycle_numbers, DKP_embeddings,
                gate_W1, gate_b1, gate_W2, gate_b2,
                expert_W, expert_b, gen_W, gen_b):
    """Host-side layout prep. Returns per-core in_maps list."""
    f32 = np.float32
    bf16 = ml_dtypes.bfloat16
    x = np.asarray(cycle_curve_data, dtype=f32).reshape(B, L, CF)

    xtf = np.empty((B, KP, L), dtype=bf16)
    xtf[:, :CF, :] = x.transpose(0, 2, 1).astype(bf16)
    xtf[:, CF, :] = np.asarray(1.0, dtype=bf16)
    xtf[:, CF + 1:, :] = np.asarray(0.0, dtype=bf16)

    ew_p = np.zeros((E, KP, D), dtype=bf16)
    ew_p[:, :CF, :] = np.asarray(expert_W, dtype=f32).astype(bf16)
    ew_p[:, CF, :] = np.asarray(expert_b, dtype=f32).astype(bf16)

    gw_p = np.zeros((KP, D), dtype=bf16)
    gw_p[:CF, :] = np.asarray(gen_W, dtype=f32).astype(bf16)
    gw_p[CF, :] = np.asarray(gen_b, dtype=f32).astype(bf16)

    gint = np.zeros((GK, B), dtype=f32)
    gint[:DLLM, :] = np.asarray(DKP_embeddings, dtype=f32).T
    gint[DLLM, :] = np.asarray(cycle_numbers, dtype=f32)[:, 0]
    gint[DLLM + 1, :] = 1.0

    w1p = np.zeros((GK, DFF), dtype=f32)
    w1p[:DLLM + 1, :] = np.asarray(gate_W1, dtype=f32)
    w1p[DLLM + 1, :] = np.asarray(gate_b1, dtype=f32)

    w2 = np.asarray(gate_W2, dtype=f32)
    b2v = np.asarray(gate_b2, dtype=f32).reshape(1, E)

    in_maps = []
    for c in range(NCORES):
        sel = np.zeros((B, BPC), dtype=f32)
        for i in range(BPC):
            sel[c * BPC + i, i] = 1.0
        in_maps.append({
            "xt": np.ascontiguousarray(xtf[c * BPC:(c + 1) * BPC]),
            "ew": ew_p,
            "gw": gw_p,
            "gint": gint,
            "w1c": np.ascontiguousarray(w1p[:, c * DFFC:(c + 1) * DFFC]),
            "w2c": np.ascontiguousarray(w2[c * DFFC:(c + 1) * DFFC, :]),
            "selt": sel,
            "b2": b2v,
        })
    return in_maps


_CACHED = {}


def run(inputs, trace=False, tmpdir=None):
    """Run on the 8 NeuronCores; returns (full_output, BassKernelResults)."""
    from concourse import bass_utils
    in_maps = prep_inputs(**inputs)
    nc = _CACHED.get("nc")
    if nc is None:
        nc = make_nc()
        _CACHED["nc"] = nc
    res = bass_utils.run_bass_kernel_spmd(
        nc, in_maps, core_ids=list(range(NCORES)), trace=trace, tmpdir=tmpdir
    )
    outs = [np.asarray(r["out"], dtype=np.float32) for r in res.results]
    full = np.concatenate(outs, axis=0)
    return full, res


def kernel(**inputs):
    full, _ = run(inputs, trace=False)
    return full
